# revision 65
# baseline (speedup 1.0000x reference)
"""Trainium2 Bass kernel for MultiHeadLocalAttention2d.

Reference computation (B=2, C=256, H=W=64, 8 heads, d=32, 7x7 window, pad 3):
    q = wq@queries + bq ; k = wk@keys + bk ; v = wv@values + bv   (1x1 convs)
    attn = softmax_{7x7 window}(q . k_patch / 16)
    out  = wo @ (attn . v_patch) + bo
Out-of-image window positions behave as score==0 (exp->1 in the softmax
denominator) with v_patch==0 (zero contribution to the numerator).

Sharding: 8 cores = 2 batches x 4 bands of 16 image rows. Each core gets its
query band plus a 22-row key/value halo band (rows outside the image are
host-zeroed) and computes its output band independently. No collectives.

Per-core layout/algorithm:
  - All matmul operands bf16 (host-cast); accumulation fp32 in PSUM.
  - Query tiles 8x8 pixels (64 queries); key halo 14x14 (pixel-padded width
    70), split into two 98-pixel chunks (7 rows x 14 cols).
  - QK computed transposed: S^T[halo_pix, q] with lhsT=k-window [32,98],
    rhs=q-tile [32,64]; 4 heads packed on the PE array via 32-row tiles.
  - exp via ScalarE (scale=1/16 fused); 7x7 window mask and 1/denominator
    applied on GPSIMD; denominator via ones-vector matmuls.
  - v^T panels (halo-pix x channel) produced directly by the v projection
    (lhsT = x_v window), so AV needs no transposes: attn_out accumulates
    into [channel, query] PSUM via 32-column PE tiles (4 heads packed).
  - v bias for in-image pixels is folded into the output projection as a
    rank-1 term: out += W2 @ s, W2[:,h] = wo[:, head h] @ bv[head h],
    s[h,q] = (denom - npad)/denom, npad = #out-of-image window positions.
"""

import functools
import os
import sys

import numpy as np

sys.path.insert(0, "/opt/trn_rl_repo")

import ml_dtypes  # noqa: E402

from concourse import bass, mybir  # noqa: E402
from concourse.tile import TileContext  # noqa: E402
from concourse.tile import add_dep_helper  # noqa: E402
from concourse.bass_utils import run_bass_kernel_spmd  # noqa: E402

BF16 = ml_dtypes.bfloat16
F32 = np.float32

B, C, H, W = 2, 256, 64, 64
HEADS, OC, D = 8, 256, 32
KW, PAD = 7, 3
BAND = 16               # query rows per core
KVR = BAND + 2 * PAD    # kv halo rows per core (22)
WP = W + 2 * PAD        # padded width (70)
NQ = BAND * W           # queries per core (1024)
NKV = KVR * W           # valid kv pixels per core (1408)
CB_N = 6016             # bf16 constant-blob columns
NTX = W // 8            # x tiles (8)
SCALE = 1.0 / 16.0      # 1/sqrt(OC)

LAST_EXEC_NS = None

AF = mybir.ActivationFunctionType
ALU = mybir.AluOpType
dt = mybir.dt


def _build_nc():
    KPHASE = int(os.environ.get("KPHASE", "3"))
    nc = bass.Bass()

    NPAN = 2 * 2 * 8 * 98  # 3136: (ty, ck, tx) panels of 98 window pixels
    xq_d = nc.dram_tensor("xq", [C, NQ], dt.bfloat16, kind="ExternalInput")
    xk_d = nc.dram_tensor("xk", [C, NKV], dt.bfloat16, kind="ExternalInput")
    xv_d = nc.dram_tensor("xv", [C, NPAN], dt.bfloat16, kind="ExternalInput")
    cblob_d = nc.dram_tensor("cblob", [128, CB_N], dt.bfloat16, kind="ExternalInput")
    bias_d = nc.dram_tensor("biases", [128, 6], dt.float32, kind="ExternalInput")
    out_d = nc.dram_tensor("out", [OC, NQ], dt.float32, kind="ExternalOutput")

    with TileContext(nc) as tc:
        with (
            tc.tile_pool(name="const", bufs=1) as const,
            tc.tile_pool(name="io", bufs=1) as io,
            tc.tile_pool(name="work", bufs=1) as work,
        ):
            # ---- constants: one bf16 blob, one fp32 blob ----
            cb = const.tile([128, CB_N], dt.bfloat16)
            nc.sync.dma_start(cb[:], cblob_d[:, :])
            wq_sb = cb[:, 0:512]
            wk_sb = cb[:, 512:1024]
            wv_sb = cb[:, 1024:1536]
            wo_sb = cb[:, 1536:2048]
            w2_sb = cb[:, 2048:2304]
            kmask_sb = cb[:, 2304:2304 + NKV]
            winm_sb = cb[:, 3712:4736]
            npad_sb = cb[:, 4736:5760]
            e4_sb = cb[:, 5760:5888]
            # sel[:, 64] = ones, else 0; slice [64-v : 128-v] = ones in col v
            sel_sb = cb[:, 5888:6016]
            bias_sb = const.tile([128, 6], dt.float32)
            nc.sync.dma_start(bias_sb[:], bias_d[:, :])
            bq_sb = bias_sb[:, 0:2]
            bk_sb = bias_sb[:, 2:4]
            bo_sb = bias_sb[:, 4:6]

            # ---- inputs ----
            xq_sb = [io.tile([128, NQ], dt.bfloat16, name=f"xq{kc}") for kc in range(2)]
            xk_sb = [io.tile([128, NKV], dt.bfloat16, name=f"xk{kc}") for kc in range(2)]
            xv_sb = [io.tile([128, NPAN], dt.bfloat16, name=f"xv{kc}") for kc in range(2)]
            for kc in range(2):
                nc.sync.dma_start(xq_sb[kc][:], xq_d[kc * 128:(kc + 1) * 128, :])
                nc.sync.dma_start(xk_sb[kc][:], xk_d[kc * 128:(kc + 1) * 128, :])
                nc.sync.dma_start(xv_sb[kc][:], xv_d[kc * 128:(kc + 1) * 128, :])

            # touch DMA'd tiles from each compute engine so later ops don't
            # need DMA waits (pre-observation), plus carrier nops
            scr = work.tile([128, 8], dt.float32, name="scr")
            nc.vector.tensor_copy(scr[:, 0:1], bias_sb[:, 0:1])
            nc.vector.tensor_copy(scr[:, 1:2], cb[:, 0:1])
            nc.scalar.copy(scr[:, 2:3], cb[:, 1:2])
            nc.gpsimd.tensor_copy(scr[:, 3:4], cb[:, 2:3])
            nc.scalar.copy(scr[:, 4:5], bias_sb[:, 1:2])
            nc.gpsimd.tensor_copy(scr[:, 5:6], bias_sb[:, 2:3])
            # wait-carrier nops for _cap_waits (each engine's early multi-wait
            # instructions shed extra waits onto these)
            for _ in range(10):
                nc.tensor.nop()
                nc.vector.nop()
                nc.scalar.nop()
                nc.gpsimd.nop()
                nc.sync.nop()

            q_sb = [work.tile([128, NQ], dt.bfloat16, name=f"q{m}") for m in range(2)]
            k_sb = [work.tile([128, KVR * WP], dt.bfloat16, name=f"k{m}") for m in range(2)]
            k_pan = [work.tile([128, NPAN], dt.bfloat16, name=f"kp{m}") for m in range(2)]
            vt_sb = work.tile([128, 16 * 2 * OC], dt.bfloat16)
            a_sb = [work.tile([128, NQ], dt.bfloat16, name=f"a{m}") for m in range(2)]
            s_sb = work.tile([128, NQ], dt.bfloat16)
            nc.gpsimd.memset(s_sb[:], 0.0)

            with (
                tc.tile_pool(name="proj_ps", bufs=2, space="PSUM") as proj_ps,
                tc.tile_pool(name="vt_ps", bufs=2, space="PSUM") as vt_ps,
            ):
                # ---- q projection -> q_sb [2][128, 1024] bf16 ----
                for mc in range(2 if KPHASE >= 1 else 0):
                    for nn in range(2):
                        qp = proj_ps.tile([128, 512], dt.float32, tag="projps", name="qp")
                        for kc in range(2):
                            nc.tensor.matmul(
                                qp[:],
                                wq_sb[:, kc * OC + mc * 128: kc * OC + (mc + 1) * 128],
                                xq_sb[kc][:, nn * 512:(nn + 1) * 512],
                                start=(kc == 0), stop=(kc == 1),
                            )
                        # xq arrives host-scrambled [(ty tx r c)], so psum and
                        # q_sb stay in QK-tile order; flat bias add
                        nc.vector.tensor_tensor(
                            q_sb[mc][:, nn * 512:(nn + 1) * 512], qp[:],
                            bq_sb[:, mc:mc + 1].to_broadcast((128, 512)),
                            ALU.add)

                # ---- k projection -> k_sb [2][128, 22, 70] bf16 padded ----
                for mc in range(2 if KPHASE >= 1 else 0):
                    nc.gpsimd.memset(k_sb[mc][:], 0.0)
                rowchunks = [(0, 8), (8, 8), (16, 6)]
                for mc in range(2 if KPHASE >= 1 else 0):
                    k3 = k_sb[mc][:].rearrange("p (r w) -> p r w", r=KVR, w=WP)
                    for (r0, nr) in rowchunks:
                        kp = proj_ps.tile([128, 512], dt.float32, tag="projps", name="kp")
                        for kc in range(2):
                            nc.tensor.matmul(
                                kp[:, 0:nr * W],
                                wk_sb[:, kc * OC + mc * 128: kc * OC + (mc + 1) * 128],
                                xk_sb[kc][:, r0 * W:(r0 + nr) * W],
                                start=(kc == 0), stop=(kc == 1),
                            )
                        # k = (k_nob + bk) * rowmask ; invalid rows -> 0
                        kt = proj_ps.tile([128, 512], dt.float32, tag="projps",
                                          name="kt")
                        nc.vector.tensor_tensor(
                            kt[:, 0:nr * W], kp[:, 0:nr * W],
                            bk_sb[:, mc:mc + 1].to_broadcast((128, nr * W)),
                            ALU.add)
                        nc.vector.tensor_tensor(
                            k3[:, r0:r0 + nr, PAD:PAD + W],
                            kt[:, 0:nr * W].rearrange("p (r w) -> p r w", r=nr, w=W),
                            kmask_sb[:, r0 * W:(r0 + nr) * W]
                            .rearrange("p (r w) -> p r w", r=nr, w=W),
                            ALU.mult)

                for _ in range(4):
                    nc.tensor.nop()
                    nc.vector.nop()
                    nc.scalar.nop()
                    nc.gpsimd.nop()
                # ---- repack k into window panels [128, (ty, ck, tx) x 98] ----
                for hg in range(2 if KPHASE >= 1 else 0):
                    for ty in range(2):
                        src = bass.AP(
                            tensor=k_sb[hg].tensor,
                            offset=8 * ty * WP,
                            ap=[[KVR * WP, 128], [7 * WP, 2], [8, 8], [WP, 7], [1, 14]],
                        )
                        dst = k_pan[hg][:, 1568 * ty:1568 * (ty + 1)].rearrange(
                            "p (ck tx wr wc) -> p ck tx wr wc", ck=2, tx=8, wr=7, wc=14)
                        if (hg + ty) % 2 == 0:
                            nc.vector.tensor_copy(dst, src)
                        else:
                            nc.gpsimd.tensor_copy(dst, src)

                # ---- v^T panels: vt_sb [98, (t, ck) x 256] bf16 ----
                # panel pixel p = 14*wr + wc, window rows (8ty+7ck)+wr, cols 8tx+wc
                for t in range(16 if KPHASE >= 1 else 0):
                    ty, tx = t // 8, t % 8
                    vp = vt_ps.tile([128, 512], dt.float32, tag="vtps")
                    for ck in range(2):
                        pan = ((ty * 2 + ck) * 8 + tx) * 98
                        for kc in range(2):
                            nc.tensor.matmul(
                                vp[0:98, ck * OC:(ck + 1) * OC],
                                xv_sb[kc][:, pan:pan + 98],
                                wv_sb[:, kc * OC:(kc + 1) * OC],
                                start=(kc == 0), stop=(kc == 1),
                            )
                    if t % 2 == 0:
                        vi = nc.vector.tensor_copy(
                            vt_sb[0:98, t * 512:(t + 1) * 512], vp[0:98, :])
                    else:
                        vi = nc.scalar.copy(
                            vt_sb[0:98, t * 512:(t + 1) * 512], vp[0:98, :])
                    if t >= 12:
                        for mk in (nc.tensor.nop(), nc.tensor.nop(),
                                   nc.scalar.nop(), nc.vector.nop(),
                                   nc.gpsimd.nop()):
                            add_dep_helper(mk.ins, vi.ins, sync=False,
                                           reason="capw carrier")

            # ---- attention ----
            if KPHASE == 1:
                for m in range(2):
                    nc.vector.tensor_copy(a_sb[m][:], q_sb[m][:])
            elif KPHASE == 0:
                for m in range(2):
                    nc.vector.tensor_copy(a_sb[m][:], xq_sb[m][:])
                    nc.vector.tensor_copy(s_sb[:], xq_sb[0][:])
            attention_on = KPHASE >= 2
            with (
                tc.tile_pool(name="att_sb", bufs=2) as att_sb,
                tc.tile_pool(name="qk_ps", bufs=1, space="PSUM") as qk_ps,
                tc.tile_pool(name="av_ps", bufs=2, space="PSUM") as av_ps,
                tc.tile_pool(name="dn_ps", bufs=1, space="PSUM") as dn_ps,
            ):
                for ty in range(2 if attention_on else 0):
                    dn = dn_ps.tile([128, 512], dt.float32, tag="dn")
                    r8 = att_sb.tile([64, 512], dt.float32, tag="r8")
                    for hg in range(2):
                        for _ in range(3):
                            nc.tensor.nop()
                            nc.vector.nop()
                            nc.scalar.nop()
                            nc.gpsimd.nop()
                        P = att_sb.tile([128, 8 * 512], dt.bfloat16, tag="P", bufs=3)
                        for ck in range(2):
                            sps = [qk_ps.tile([128, 512], dt.float32, tag=f"sps{g}",
                                              name=f"sps{g}") for g in range(4)]
                            for tx in range(8):
                                for g in range(4):
                                    pan = ((ty * 2 + ck) * 8 + tx) * 98
                                    nc.tensor.matmul(
                                        sps[g][0:98, 64 * tx:64 * tx + 64],
                                        k_pan[hg][32 * g:32 * g + 32, pan:pan + 98],
                                        q_sb[hg][32 * g:32 * g + 32,
                                                 512 * ty + 64 * tx:
                                                 512 * ty + 64 * tx + 64],
                                        start=True, stop=True,
                                        tile_position=(32 * g, 0),
                                    )
                            for g in range(4):
                                ei = nc.scalar.activation(
                                    P[0:98, (2 * g + ck) * 512:(2 * g + ck + 1) * 512],
                                    sps[g][0:98, :], AF.Exp, scale=SCALE)
                                mi = nc.gpsimd.tensor_tensor(
                                    P[0:98, (2 * g + ck) * 512:(2 * g + ck + 1) * 512],
                                    P[0:98, (2 * g + ck) * 512:(2 * g + ck + 1) * 512],
                                    winm_sb[0:98, ck * 512:(ck + 1) * 512],
                                    ALU.mult)
                                for anch in (ei, mi):
                                    for mk in (nc.scalar.nop(), nc.gpsimd.nop(),
                                               nc.vector.nop(), nc.tensor.nop()):
                                        add_dep_helper(mk.ins, anch.ins, sync=False,
                                                       reason="capw carrier")
                        # denominators: head h=4hg+g -> dn row 32hg+g (selector
                        # lhsT puts head h's sum there, zeros elsewhere; one
                        # accumulation group spans both hg passes)
                        for g in range(4):
                            v = 32 * hg + g
                            for ck in range(2):
                                nc.tensor.matmul(
                                    dn[0:64, :],
                                    sel_sb[0:98, 64 - v:128 - v],
                                    P[0:98, (2 * g + ck) * 512:(2 * g + ck + 1) * 512],
                                    start=(hg == 0 and g == 0 and ck == 0),
                                    stop=(hg == 1 and g == 3 and ck == 1),
                                    tile_position=(0, 0),
                                )
                        nc.vector.reciprocal(r8[32 * hg:32 * hg + 4, :],
                                             dn[32 * hg:32 * hg + 4, :])
                        # rx[p, q] = 1/denom[head(p), q] via K=4 indicator matmul
                        r4b = att_sb.tile([64, 512], dt.bfloat16, tag="r4b")
                        nc.vector.tensor_copy(r4b[32 * hg:32 * hg + 4, :],
                                              r8[32 * hg:32 * hg + 4, :])
                        rxp = dn_ps.tile([128, 512], dt.float32, tag="rx")
                        nc.tensor.matmul(
                            rxp[:], e4_sb[32 * hg:32 * hg + 4, :],
                            r4b[32 * hg:32 * hg + 4, :],
                            start=True, stop=True, tile_position=(32 * hg, 0))
                        rx = att_sb.tile([128, 512], dt.bfloat16, tag="rxsb")
                        nc.vector.tensor_copy(rx[:], rxp[:])
                        # AV: attn_out [128ch, 512q] for this (ty, hg)
                        av = av_ps.tile([128, 512], dt.float32, tag="av")
                        for tx in range(8):
                            for g in range(4):
                                for ck in range(2):
                                    t = 8 * ty + tx
                                    nc.tensor.matmul(
                                        av[32 * g:32 * g + 32, 64 * tx:64 * tx + 64],
                                        vt_sb[0:98,
                                              t * 512 + ck * OC + hg * 128 + 32 * g:
                                              t * 512 + ck * OC + hg * 128 + 32 * g + 32],
                                        P[0:98, (2 * g + ck) * 512 + 64 * tx:
                                          (2 * g + ck) * 512 + 64 * tx + 64],
                                        start=(ck == 0), stop=(ck == 1),
                                        tile_position=(0, 32 * g),
                                    )
                        # evacuate + normalize + unscramble (tx,r,c) -> (y,x)
                        a4 = a_sb[hg][:].rearrange(
                            "p (ty r xt c) -> p ty xt r c", ty=2, r=8, xt=8, c=8)
                        nc.vector.tensor_tensor(
                            a4[:, ty],
                            av[:].rearrange("p (xt r c) -> p xt r c", xt=8, r=8, c=8),
                            rx[:].rearrange("p (xt r c) -> p xt r c", xt=8, r=8, c=8),
                            ALU.mult)
                        # s[h, q] = 1 - npad/denom (v-bias weight, head h at
                        # partition row 32hg+g to keep all ops lane-aligned)
                        t8 = att_sb.tile([64, 512], dt.float32, tag="t8")
                        np_s = npad_sb[32 * hg:32 * hg + 4, :].rearrange(
                            "p (ty f) -> p ty f", ty=2)[:, ty]
                        nc.vector.tensor_tensor(
                            t8[32 * hg:32 * hg + 4, :],
                            r8[32 * hg:32 * hg + 4, :], np_s, ALU.mult)
                        s4 = s_sb[:].rearrange(
                            "p (ty r xt c) -> p ty xt r c", ty=2, r=8, xt=8, c=8)
                        nc.vector.tensor_scalar(
                            s4[32 * hg:32 * hg + 4, ty],
                            t8[32 * hg:32 * hg + 4, :].rearrange(
                                "p (xt r c) -> p xt r c", xt=8, r=8, c=8),
                            -1.0, 1.0, ALU.mult, ALU.add)

            # ---- output projection ----
            out_sb = [work.tile([128, NQ], dt.float32, name=f"o{m}") for m in range(2)]
            with tc.tile_pool(name="oproj_ps", bufs=2, space="PSUM") as oproj_ps:
                for mc in range(2):
                    for nn in range(2):
                        op = oproj_ps.tile([128, 512], dt.float32, tag="ops", name="op")
                        for kc in range(2):
                            nc.tensor.matmul(
                                op[:],
                                wo_sb[:, kc * OC + mc * 128: kc * OC + (mc + 1) * 128],
                                a_sb[kc][:, nn * 512:(nn + 1) * 512],
                                start=(kc == 0), stop=False,
                            )
                        nc.tensor.matmul(
                            op[:],
                            w2_sb[:, mc * 128:(mc + 1) * 128],
                            s_sb[:, nn * 512:(nn + 1) * 512],
                            start=False, stop=True,
                        )
                        nc.vector.tensor_tensor(
                            out_sb[mc][:, nn * 512:(nn + 1) * 512], op[:],
                            bo_sb[:, mc:mc + 1].to_broadcast((128, 512)),
                            ALU.add)
            last_dma = None
            for mc in range(2):
                last_dma = nc.sync.dma_start(
                    out_d[mc * 128:(mc + 1) * 128, :], out_sb[mc][:])
            for _ in range(24):
                for mk in (nc.sync.nop(), nc.vector.nop(), nc.scalar.nop(),
                           nc.tensor.nop(), nc.gpsimd.nop()):
                    add_dep_helper(mk.ins, last_dma.ins, sync=False,
                                   reason="capw tail carrier")

    return nc


def _cap_waits(nc):
    """This walrus build accepts at most one attached sync wait per
    instruction. Move extra waits onto preceding same-engine instructions
    that (a) have a free wait slot and (b) are positioned after the wait's
    producer in the scheduled order, so no new wait-for cycles can form."""
    for f in nc.m.functions:
        blocks = list(f.blocks)
        glob = []  # (bi, idx, ins) in scheduled order
        for bi, bb in enumerate(blocks):
            for idx, ins in enumerate(bb.instructions):
                glob.append((bi, idx, ins))
        # cumulative semaphore values by global position
        sem_hist = {}  # sem_id -> list of (gpos, cum_after)
        cum = {}
        for g, (bi, idx, ins) in enumerate(glob):
            si = ins.sync_info
            if si is None:
                continue
            for u in si.on_update:
                cum[u.id] = cum.get(u.id, 0) + (u.update_value or 1)
                sem_hist.setdefault(u.id, []).append((g, cum[u.id]))

        def producer_pos(w):
            hist = sem_hist.get(w.id)
            if hist is None:
                return -1
            v = w.wait_value or 1
            for g, c in hist:
                if c >= v:
                    return g
            return len(glob)  # never satisfied: don't move

        # drop waits on the holder's own engine counter: compute engines
        # complete in order, so program order already implies them
        eng_sem = {"EngineType.PE": "PE", "EngineType.DVE": "DVE",
                   "EngineType.Activation": "Activation",
                   "EngineType.Pool": "Pool", "EngineType.SP": "SP"}
        for g, (bi, idx, ins) in enumerate(glob):
            si = ins.sync_info
            if si is None or not si.on_wait:
                continue
            own = eng_sem.get(str(ins.engine))
            kept = [w for w in si.on_wait
                    if not (own and w.ant_name.startswith(own + "_"))]
            if len(kept) != len(si.on_wait):
                ins.sync_info = mybir.SyncInfo(
                    on_wait=kept, on_update=list(si.on_update))

        nwaits = [0 if i.sync_info is None else len(i.sync_info.on_wait)
                  for (_, _, i) in glob]
        eng_pos = {}  # engine -> [gpos...]
        for g, (bi, idx, ins) in enumerate(glob):
            eng_pos.setdefault(str(ins.engine), []).append(g)

        for g, (bi, idx, ins) in enumerate(glob):
            si = ins.sync_info
            if si is None or len(si.on_wait) <= 1:
                continue
            waits = sorted(si.on_wait, key=producer_pos, reverse=True)
            keep = [waits.pop(0)]  # latest-satisfied stays attached
            leftovers = []
            idxs = eng_pos[str(ins.engine)]
            mypos = idxs.index(g)
            for w in waits:
                placed = False
                pmin = producer_pos(w)
                for pp in range(mypos - 1, -1, -1):
                    cg = idxs[pp]
                    if cg <= pmin:
                        break  # earlier carriers are all unsafe
                    cins = glob[cg][2]
                    if nwaits[cg] == 0 and type(cins).__name__ in (
                            "InstNoOp", "InstDrain", "InstMemset", "InstCopy",
                            "InstTensorTensor", "InstActivation",
                            "InstTensorScalarPtr", "InstReciprocal",
                            "InstMatmult", "InstTensorReduce", "InstDMACopy"):
                        csi = cins.sync_info
                        upd = [] if csi is None else list(csi.on_update)
                        cins.sync_info = mybir.SyncInfo(
                            on_wait=[w], on_update=upd)
                        nwaits[cg] = 1
                        placed = True
                        break
                if not placed:
                    leftovers.append(w)
            keep.extend(leftovers)
            ins.sync_info = mybir.SyncInfo(
                on_wait=keep, on_update=list(si.on_update))
            nwaits[g] = len(keep)


@functools.lru_cache(maxsize=1)
def _get_nc():
    nc = _build_nc()
    _cap_waits(nc)
    return nc


def _host_inputs(queries, keys, values, wq, bq, wk, bk, wv, bv, wo, bo):
    """Build the 8 per-core input maps."""
    # window mask [98, 1024]: p = 14*wr+wc (chunk ck), qf = 8r+c, tiled x8 tx
    winm = np.zeros((2, 98, 64), dtype=F32)
    for ck in range(2):
        for wr in range(7):
            for wc in range(14):
                for r in range(8):
                    for c in range(8):
                        dy = (7 * ck + wr) - 3 - r
                        dx = wc - 3 - c
                        if abs(dy) <= 3 and abs(dx) <= 3:
                            winm[ck, 14 * wr + wc, 8 * r + c] = 1.0
    winm = np.concatenate([np.tile(winm[ck], (1, 8)) for ck in range(2)], axis=1)
    winm = np.ascontiguousarray(winm).astype(BF16)  # [98, 1024]

    # w2 row (32*(h//4) + h%4) = wo[:, head h] @ bv[head h]
    w2 = np.zeros((128, OC), dtype=F32)
    for h in range(HEADS):
        sl = slice(32 * h, 32 * h + 32)
        w2[32 * (h // 4) + h % 4] = wo[:, sl] @ bv[sl]
    e4 = np.zeros((128, 128), dtype=F32)
    for g in range(4):
        e4[g, 32 * g:32 * g + 32] = 1.0
        e4[32 + g, 32 * g:32 * g + 32] = 1.0
    sel = np.zeros((128, 128), dtype=F32)
    sel[0:98, 64] = 1.0

    def wblk(w):  # [256, 256] -> [128p, (kc 2) x 256oc] kc-major
        t = np.ascontiguousarray(w.T).reshape(2, 128, OC)
        return np.concatenate([t[0], t[1]], axis=1)

    biases = np.zeros((128, 6), dtype=F32)
    for t, barr in enumerate((bq, bk, bo)):
        biases[:, 2 * t:2 * t + 2] = barr.reshape(2, 128).T

    in_maps = []
    for core in range(8):
        b, band = core // 4, core % 4
        y0 = band * BAND
        xq = queries[b, :, y0:y0 + BAND, :].reshape(C, 2, 8, 8, 8)
        xq = np.ascontiguousarray(
            xq.transpose(0, 1, 3, 2, 4)).reshape(C, NQ).astype(BF16)
        xk = np.zeros((C, KVR, W), dtype=F32)
        xv = np.zeros((C, KVR, W + 2 * PAD), dtype=F32)
        lo, hi = y0 - PAD, y0 + BAND + PAD
        slo, shi = max(lo, 0), min(hi, H)
        xk[:, slo - lo:shi - lo, :] = keys[b, :, slo:shi, :]
        xv[:, slo - lo:shi - lo, PAD:PAD + W] = values[b, :, slo:shi, :]
        kmask = np.zeros((1, KVR, W), dtype=F32)
        kmask[:, slo - lo:shi - lo, :] = 1.0
        # xv panels: [(ty, ck, tx), wr, wc] window pixels, contiguous per panel
        xvp = np.zeros((C, 2, 2, 8, 7, 14), dtype=F32)
        for tty in range(2):
            for ck in range(2):
                r0 = 8 * tty + 7 * ck
                for tx in range(8):
                    xvp[:, tty, ck, tx] = xv[:, r0:r0 + 7, 8 * tx:8 * tx + 14]
        # npad[q] = # window positions outside the image, scrambled order
        npad = np.zeros((2, 8, 8, 8), dtype=F32)  # [ty, tx, r, c]
        for tty in range(2):
            for tx in range(8):
                for r in range(8):
                    for c in range(8):
                        y = y0 + 8 * tty + r
                        x = 8 * tx + c
                        ny = np.clip(np.arange(y - 3, y + 4), -1, H)
                        nx = np.clip(np.arange(x - 3, x + 4), -1, W)
                        inside = ((ny >= 0) & (ny < H))[:, None] & ((nx >= 0) & (nx < W))[None, :]
                        npad[tty, tx, r, c] = 49 - inside.sum()
        npad128 = np.zeros((128, NQ), dtype=F32)
        npad128[0:4] = npad.reshape(1, NQ)
        npad128[32:36] = npad.reshape(1, NQ)

        cblob = np.zeros((128, CB_N), dtype=F32)
        cblob[:, 0:512] = wblk(wq)
        cblob[:, 512:1024] = wblk(wk)
        cblob[:, 1024:1536] = wblk(wv)
        cblob[:, 1536:2048] = wblk(wo)
        cblob[:, 2048:2304] = w2
        cblob[:, 2304:2304 + NKV] = np.broadcast_to(kmask.reshape(1, NKV), (128, NKV))
        cblob[0:98, 3712:4736] = winm
        cblob[:, 4736:5760] = npad128
        cblob[:, 5760:5888] = e4
        cblob[:, 5888:6016] = sel
        in_maps.append(dict(
            xq=xq,
            xk=xk.reshape(C, NKV).astype(BF16),
            xv=xvp.reshape(C, 2 * 2 * 8 * 98).astype(BF16),
            cblob=cblob.astype(BF16),
            biases=biases,
        ))
    return in_maps


def kernel(queries, keys, values, wq, bq, wk, bk, wv, bv, wo, bo):
    global LAST_EXEC_NS
    nc = _get_nc()
    in_maps = _host_inputs(queries, keys, values, wq, bq, wk, bk, wv, bv, wo, bo)
    trace = bool(os.environ.get("KERNEL_TRACE"))
    try:
        res = run_bass_kernel_spmd(nc, in_maps, core_ids=list(range(8)),
                                   trace=trace)
    except ModuleNotFoundError:
        # NTFF profile hook unavailable in this container
        res = run_bass_kernel_spmd(nc, in_maps, core_ids=list(range(8)),
                                   trace=False)
    LAST_EXEC_NS = res.exec_time_ns
    out = np.zeros((B, OC, H, W), dtype=F32)
    for core in range(8):
        b, band = core // 4, core % 4
        y0 = band * BAND
        out[b, :, y0:y0 + BAND, :] = res.results[core]["out"].reshape(OC, BAND, W)
    return out


# revision 66
# speedup vs baseline: 1.1647x; 1.1647x over previous
"""Trainium2 Bass kernel for MultiHeadLocalAttention2d.

Reference computation (B=2, C=256, H=W=64, 8 heads, d=32, 7x7 window, pad 3):
    q = wq@queries + bq ; k = wk@keys + bk ; v = wv@values + bv   (1x1 convs)
    attn = softmax_{7x7 window}(q . k_patch / 16)
    out  = wo @ (attn . v_patch) + bo
Out-of-image window positions behave as score==0 (exp->1 in the softmax
denominator) with v_patch==0 (zero contribution to the numerator).

Sharding: 8 cores = 2 batches x 4 bands of 16 image rows. Each core gets its
query band plus a 22-row key/value halo band (rows outside the image are
host-zeroed) and computes its output band independently. No collectives.

Per-core layout/algorithm:
  - All matmul operands bf16 (host-cast); accumulation fp32 in PSUM.
  - Query tiles 8x8 pixels (64 queries); key halo 14x14 (pixel-padded width
    70), split into two 98-pixel chunks (7 rows x 14 cols).
  - QK computed transposed: S^T[halo_pix, q] with lhsT=k-window [32,98],
    rhs=q-tile [32,64]; 4 heads packed on the PE array via 32-row tiles.
  - exp via ScalarE (scale=1/16 fused); 7x7 window mask and 1/denominator
    applied on GPSIMD; denominator via ones-vector matmuls.
  - v^T panels (halo-pix x channel) produced directly by the v projection
    (lhsT = x_v window), so AV needs no transposes: attn_out accumulates
    into [channel, query] PSUM via 32-column PE tiles (4 heads packed).
  - v bias for in-image pixels is folded into the output projection as a
    rank-1 term: out += W2 @ s, W2[:,h] = wo[:, head h] @ bv[head h],
    s[h,q] = (denom - npad)/denom, npad = #out-of-image window positions.
"""

import functools
import os
import sys

import numpy as np

sys.path.insert(0, "/opt/trn_rl_repo")

import ml_dtypes  # noqa: E402

from concourse import bass, mybir  # noqa: E402
from concourse.tile import TileContext  # noqa: E402
from concourse.tile import add_dep_helper  # noqa: E402
from concourse.bass_utils import run_bass_kernel_spmd  # noqa: E402

BF16 = ml_dtypes.bfloat16
F32 = np.float32

B, C, H, W = 2, 256, 64, 64
HEADS, OC, D = 8, 256, 32
KW, PAD = 7, 3
BAND = 16               # query rows per core
KVR = BAND + 2 * PAD    # kv halo rows per core (22)
WP = W + 2 * PAD        # padded width (70)
NQ = BAND * W           # queries per core (1024)
NKV = KVR * W           # valid kv pixels per core (1408)
CB_N = 6016             # bf16 constant-blob columns
NTX = W // 8            # x tiles (8)
SCALE = 1.0 / 16.0      # 1/sqrt(OC)

LAST_EXEC_NS = None

AF = mybir.ActivationFunctionType
ALU = mybir.AluOpType
dt = mybir.dt


def _build_nc():
    KPHASE = int(os.environ.get("KPHASE", "3"))
    nc = bass.Bass()

    NPAN = 2 * 2 * 8 * 98  # 3136: (ty, ck, tx) panels of 98 window pixels
    xq_d = nc.dram_tensor("xq", [C, NQ], dt.bfloat16, kind="ExternalInput")
    xk_d = nc.dram_tensor("xk", [C, NKV], dt.bfloat16, kind="ExternalInput")
    xv_d = nc.dram_tensor("xv", [C, NPAN], dt.bfloat16, kind="ExternalInput")
    cblob_d = nc.dram_tensor("cblob", [128, CB_N], dt.bfloat16, kind="ExternalInput")
    bias_d = nc.dram_tensor("biases", [128, 6], dt.float32, kind="ExternalInput")
    out_d = nc.dram_tensor("out", [OC, NQ], dt.float32, kind="ExternalOutput")

    with TileContext(nc) as tc:
        with (
            tc.tile_pool(name="const", bufs=1) as const,
            tc.tile_pool(name="io", bufs=1) as io,
            tc.tile_pool(name="work", bufs=1) as work,
        ):
            # ---- constants: one bf16 blob, one fp32 blob ----
            cb = const.tile([128, CB_N], dt.bfloat16)
            nc.sync.dma_start(cb[:], cblob_d[:, :])
            wq_sb = cb[:, 0:512]
            wk_sb = cb[:, 512:1024]
            wv_sb = cb[:, 1024:1536]
            wo_sb = cb[:, 1536:2048]
            w2_sb = cb[:, 2048:2304]
            kmask_sb = cb[:, 2304:2304 + NKV]
            winm_sb = cb[:, 3712:4736]
            npad_sb = cb[:, 4736:5760]
            e4_sb = cb[:, 5760:5888]
            # sel[:, 64] = ones, else 0; slice [64-v : 128-v] = ones in col v
            sel_sb = cb[:, 5888:6016]
            bias_sb = const.tile([128, 6], dt.float32)
            nc.sync.dma_start(bias_sb[:], bias_d[:, :])
            bq_sb = bias_sb[:, 0:2]
            bk_sb = bias_sb[:, 2:4]
            bo_sb = bias_sb[:, 4:6]

            # ---- inputs ----
            xq_sb = [io.tile([128, NQ], dt.bfloat16, name=f"xq{kc}") for kc in range(2)]
            xk_sb = [io.tile([128, NKV], dt.bfloat16, name=f"xk{kc}") for kc in range(2)]
            xv_sb = [io.tile([128, NPAN], dt.bfloat16, name=f"xv{kc}") for kc in range(2)]
            for kc in range(2):
                nc.sync.dma_start(xq_sb[kc][:], xq_d[kc * 128:(kc + 1) * 128, :])
                nc.sync.dma_start(xk_sb[kc][:], xk_d[kc * 128:(kc + 1) * 128, :])
                nc.sync.dma_start(xv_sb[kc][:], xv_d[kc * 128:(kc + 1) * 128, :])

            # touch DMA'd tiles from each compute engine so later ops don't
            # need DMA waits (pre-observation), plus carrier nops
            scr = work.tile([128, 8], dt.float32, name="scr")
            nc.vector.tensor_copy(scr[:, 0:1], bias_sb[:, 0:1])
            nc.vector.tensor_copy(scr[:, 1:2], cb[:, 0:1])
            nc.scalar.copy(scr[:, 2:3], cb[:, 1:2])
            nc.gpsimd.tensor_copy(scr[:, 3:4], cb[:, 2:3])
            nc.scalar.copy(scr[:, 4:5], bias_sb[:, 1:2])
            nc.gpsimd.tensor_copy(scr[:, 5:6], bias_sb[:, 2:3])
            # wait-carrier nops for _cap_waits (each engine's early multi-wait
            # instructions shed extra waits onto these)
            for _ in range(10):
                nc.tensor.nop()
                nc.vector.nop()
                nc.scalar.nop()
                nc.gpsimd.nop()
                nc.sync.nop()

            q_sb = [work.tile([128, NQ], dt.bfloat16, name=f"q{m}") for m in range(2)]
            k_sb = [work.tile([128, KVR * WP], dt.bfloat16, name=f"k{m}") for m in range(2)]
            k_pan = [work.tile([128, NPAN], dt.bfloat16, name=f"kp{m}") for m in range(2)]
            vt_sb = work.tile([128, 16 * 2 * OC], dt.bfloat16)
            a_sb = [work.tile([128, NQ], dt.bfloat16, name=f"a{m}") for m in range(2)]
            s_sb = work.tile([128, NQ], dt.bfloat16)
            nc.gpsimd.memset(s_sb[:], 0.0)

            with (
                tc.tile_pool(name="proj_ps", bufs=2, space="PSUM") as proj_ps,
                tc.tile_pool(name="vt_ps", bufs=2, space="PSUM") as vt_ps,
            ):
                # ---- q projection -> q_sb [2][128, 1024] bf16 ----
                for mc in range(2 if KPHASE >= 1 else 0):
                    for nn in range(2):
                        qp = proj_ps.tile([128, 512], dt.float32, tag="projps", name="qp")
                        for kc in range(2):
                            nc.tensor.matmul(
                                qp[:],
                                wq_sb[:, kc * OC + mc * 128: kc * OC + (mc + 1) * 128],
                                xq_sb[kc][:, nn * 512:(nn + 1) * 512],
                                start=(kc == 0), stop=(kc == 1),
                            )
                        # xq arrives host-scrambled [(ty tx r c)], so psum and
                        # q_sb stay in QK-tile order; flat bias add
                        nc.vector.tensor_tensor(
                            q_sb[mc][:, nn * 512:(nn + 1) * 512], qp[:],
                            bq_sb[:, mc:mc + 1].to_broadcast((128, 512)),
                            ALU.add)

                # ---- k projection -> k_sb [2][128, 22, 70] bf16 padded ----
                for mc in range(2 if KPHASE >= 1 else 0):
                    nc.gpsimd.memset(k_sb[mc][:], 0.0)
                rowchunks = [(0, 8), (8, 8), (16, 6)]
                for mc in range(2 if KPHASE >= 1 else 0):
                    k3 = k_sb[mc][:].rearrange("p (r w) -> p r w", r=KVR, w=WP)
                    for (r0, nr) in rowchunks:
                        kp = proj_ps.tile([128, 512], dt.float32, tag="projps", name="kp")
                        for kc in range(2):
                            nc.tensor.matmul(
                                kp[:, 0:nr * W],
                                wk_sb[:, kc * OC + mc * 128: kc * OC + (mc + 1) * 128],
                                xk_sb[kc][:, r0 * W:(r0 + nr) * W],
                                start=(kc == 0), stop=(kc == 1),
                            )
                        # k = (k_nob + bk) * rowmask ; invalid rows -> 0
                        kt = proj_ps.tile([128, 512], dt.float32, tag="projps",
                                          name="kt")
                        nc.vector.tensor_tensor(
                            kt[:, 0:nr * W], kp[:, 0:nr * W],
                            bk_sb[:, mc:mc + 1].to_broadcast((128, nr * W)),
                            ALU.add)
                        nc.vector.tensor_tensor(
                            k3[:, r0:r0 + nr, PAD:PAD + W],
                            kt[:, 0:nr * W].rearrange("p (r w) -> p r w", r=nr, w=W),
                            kmask_sb[:, r0 * W:(r0 + nr) * W]
                            .rearrange("p (r w) -> p r w", r=nr, w=W),
                            ALU.mult)

                for _ in range(4):
                    nc.tensor.nop()
                    nc.vector.nop()
                    nc.scalar.nop()
                    nc.gpsimd.nop()
                # ---- repack k into window panels [128, (ty, ck, tx) x 98] ----
                for hg in range(2 if KPHASE >= 1 else 0):
                    for ty in range(2):
                        src = bass.AP(
                            tensor=k_sb[hg].tensor,
                            offset=8 * ty * WP,
                            ap=[[KVR * WP, 128], [7 * WP, 2], [8, 8], [WP, 7], [1, 14]],
                        )
                        dst = k_pan[hg][:, 1568 * ty:1568 * (ty + 1)].rearrange(
                            "p (ck tx wr wc) -> p ck tx wr wc", ck=2, tx=8, wr=7, wc=14)
                        if (hg + ty) % 2 == 0:
                            nc.vector.tensor_copy(dst, src)
                        else:
                            nc.gpsimd.tensor_copy(dst, src)

                # ---- v^T panels: vt_sb [98, (t, ck) x 256] bf16 ----
                # panel pixel p = 14*wr + wc, window rows (8ty+7ck)+wr, cols 8tx+wc
                for t in range(16 if KPHASE >= 1 else 0):
                    ty, tx = t // 8, t % 8
                    vp = vt_ps.tile([128, 512], dt.float32, tag="vtps")
                    for ck in range(2):
                        pan = ((ty * 2 + ck) * 8 + tx) * 98
                        for kc in range(2):
                            nc.tensor.matmul(
                                vp[0:98, ck * OC:(ck + 1) * OC],
                                xv_sb[kc][:, pan:pan + 98],
                                wv_sb[:, kc * OC:(kc + 1) * OC],
                                start=(kc == 0), stop=(kc == 1),
                            )
                    if t % 2 == 0:
                        vi = nc.vector.tensor_copy(
                            vt_sb[0:98, t * 512:(t + 1) * 512], vp[0:98, :])
                    else:
                        vi = nc.scalar.copy(
                            vt_sb[0:98, t * 512:(t + 1) * 512], vp[0:98, :])
                    if t >= 12:
                        for mk in (nc.tensor.nop(), nc.tensor.nop(),
                                   nc.scalar.nop(), nc.vector.nop(),
                                   nc.gpsimd.nop()):
                            add_dep_helper(mk.ins, vi.ins, sync=False,
                                           reason="capw carrier")

            # ---- attention ----
            if KPHASE == 1:
                for m in range(2):
                    nc.vector.tensor_copy(a_sb[m][:], q_sb[m][:])
            elif KPHASE == 0:
                for m in range(2):
                    nc.vector.tensor_copy(a_sb[m][:], xq_sb[m][:])
                    nc.vector.tensor_copy(s_sb[:], xq_sb[0][:])
            attention_on = KPHASE >= 2
            with (
                tc.tile_pool(name="att_sb", bufs=2) as att_sb,
                tc.tile_pool(name="qk_ps", bufs=1, space="PSUM") as qk_ps,
                tc.tile_pool(name="av_ps", bufs=2, space="PSUM") as av_ps,
                tc.tile_pool(name="dn_ps", bufs=1, space="PSUM") as dn_ps,
            ):
                for ty in range(2 if attention_on else 0):
                    dn = dn_ps.tile([128, 512], dt.float32, tag="dn")
                    r8 = att_sb.tile([64, 512], dt.float32, tag="r8")
                    for hg in range(2):
                        for _ in range(3):
                            nc.tensor.nop()
                            nc.vector.nop()
                            nc.scalar.nop()
                            nc.gpsimd.nop()
                        P = att_sb.tile([128, 8 * 512], dt.bfloat16, tag="P", bufs=3)
                        for ck in range(2):
                            sps = [qk_ps.tile([128, 512], dt.float32, tag=f"sps{g}",
                                              name=f"sps{g}") for g in range(4)]
                            for tx in range(8):
                                for g in range(4):
                                    pan = ((ty * 2 + ck) * 8 + tx) * 98
                                    nc.tensor.matmul(
                                        sps[g][0:98, 64 * tx:64 * tx + 64],
                                        k_pan[hg][32 * g:32 * g + 32, pan:pan + 98],
                                        q_sb[hg][32 * g:32 * g + 32,
                                                 512 * ty + 64 * tx:
                                                 512 * ty + 64 * tx + 64],
                                        start=True, stop=True,
                                        tile_position=(32 * g, 0),
                                    )
                            for g in range(4):
                                ei = nc.scalar.activation(
                                    P[0:98, (2 * g + ck) * 512:(2 * g + ck + 1) * 512],
                                    sps[g][0:98, :], AF.Exp, scale=SCALE)
                                mi = nc.gpsimd.tensor_tensor(
                                    P[0:98, (2 * g + ck) * 512:(2 * g + ck + 1) * 512],
                                    P[0:98, (2 * g + ck) * 512:(2 * g + ck + 1) * 512],
                                    winm_sb[0:98, ck * 512:(ck + 1) * 512],
                                    ALU.mult)
                                for anch in (ei, mi):
                                    for mk in (nc.scalar.nop(), nc.gpsimd.nop(),
                                               nc.vector.nop(), nc.tensor.nop()):
                                        add_dep_helper(mk.ins, anch.ins, sync=False,
                                                       reason="capw carrier")
                        # denominators: head h=4hg+g -> dn row 32hg+g (selector
                        # lhsT puts head h's sum there, zeros elsewhere; one
                        # accumulation group spans both hg passes)
                        for g in range(4):
                            v = 32 * hg + g
                            for ck in range(2):
                                nc.tensor.matmul(
                                    dn[0:64, :],
                                    sel_sb[0:98, 64 - v:128 - v],
                                    P[0:98, (2 * g + ck) * 512:(2 * g + ck + 1) * 512],
                                    start=(hg == 0 and g == 0 and ck == 0),
                                    stop=(hg == 1 and g == 3 and ck == 1),
                                    tile_position=(0, 0),
                                )
                        nc.vector.reciprocal(r8[32 * hg:32 * hg + 4, :],
                                             dn[32 * hg:32 * hg + 4, :])
                        # rx[p, q] = 1/denom[head(p), q] via K=4 indicator matmul
                        r4b = att_sb.tile([64, 512], dt.bfloat16, tag="r4b")
                        nc.vector.tensor_copy(r4b[32 * hg:32 * hg + 4, :],
                                              r8[32 * hg:32 * hg + 4, :])
                        rxp = dn_ps.tile([128, 512], dt.float32, tag="rx")
                        nc.tensor.matmul(
                            rxp[:], e4_sb[32 * hg:32 * hg + 4, :],
                            r4b[32 * hg:32 * hg + 4, :],
                            start=True, stop=True, tile_position=(32 * hg, 0))
                        rx = att_sb.tile([128, 512], dt.bfloat16, tag="rxsb")
                        nc.vector.tensor_copy(rx[:], rxp[:])
                        # AV: attn_out [128ch, 512q] for this (ty, hg)
                        av = av_ps.tile([128, 512], dt.float32, tag="av")
                        for tx in range(8):
                            for g in range(4):
                                for ck in range(2):
                                    t = 8 * ty + tx
                                    nc.tensor.matmul(
                                        av[32 * g:32 * g + 32, 64 * tx:64 * tx + 64],
                                        vt_sb[0:98,
                                              t * 512 + ck * OC + hg * 128 + 32 * g:
                                              t * 512 + ck * OC + hg * 128 + 32 * g + 32],
                                        P[0:98, (2 * g + ck) * 512 + 64 * tx:
                                          (2 * g + ck) * 512 + 64 * tx + 64],
                                        start=(ck == 0), stop=(ck == 1),
                                        tile_position=(0, 32 * g),
                                    )
                        # evacuate + normalize + unscramble (tx,r,c) -> (y,x)
                        a4 = a_sb[hg][:].rearrange(
                            "p (ty r xt c) -> p ty xt r c", ty=2, r=8, xt=8, c=8)
                        nc.vector.tensor_tensor(
                            a4[:, ty],
                            av[:].rearrange("p (xt r c) -> p xt r c", xt=8, r=8, c=8),
                            rx[:].rearrange("p (xt r c) -> p xt r c", xt=8, r=8, c=8),
                            ALU.mult)
                        # s[h, q] = 1 - npad/denom (v-bias weight, head h at
                        # partition row 32hg+g to keep all ops lane-aligned)
                        t8 = att_sb.tile([64, 512], dt.float32, tag="t8")
                        np_s = npad_sb[32 * hg:32 * hg + 4, :].rearrange(
                            "p (ty f) -> p ty f", ty=2)[:, ty]
                        nc.vector.tensor_tensor(
                            t8[32 * hg:32 * hg + 4, :],
                            r8[32 * hg:32 * hg + 4, :], np_s, ALU.mult)
                        s4 = s_sb[:].rearrange(
                            "p (ty r xt c) -> p ty xt r c", ty=2, r=8, xt=8, c=8)
                        nc.vector.tensor_scalar(
                            s4[32 * hg:32 * hg + 4, ty],
                            t8[32 * hg:32 * hg + 4, :].rearrange(
                                "p (xt r c) -> p xt r c", xt=8, r=8, c=8),
                            -1.0, 1.0, ALU.mult, ALU.add)

            # ---- output projection ----
            out_sb = [work.tile([128, NQ], dt.float32, name=f"o{m}") for m in range(2)]
            with tc.tile_pool(name="oproj_ps", bufs=2, space="PSUM") as oproj_ps:
                for mc in range(2):
                    for nn in range(2):
                        op = oproj_ps.tile([128, 512], dt.float32, tag="ops", name="op")
                        for kc in range(2):
                            nc.tensor.matmul(
                                op[:],
                                wo_sb[:, kc * OC + mc * 128: kc * OC + (mc + 1) * 128],
                                a_sb[kc][:, nn * 512:(nn + 1) * 512],
                                start=(kc == 0), stop=False,
                            )
                        nc.tensor.matmul(
                            op[:],
                            w2_sb[:, mc * 128:(mc + 1) * 128],
                            s_sb[:, nn * 512:(nn + 1) * 512],
                            start=False, stop=True,
                        )
                        nc.vector.tensor_tensor(
                            out_sb[mc][:, nn * 512:(nn + 1) * 512], op[:],
                            bo_sb[:, mc:mc + 1].to_broadcast((128, 512)),
                            ALU.add)
            last_dma = None
            for mc in range(2):
                last_dma = nc.sync.dma_start(
                    out_d[mc * 128:(mc + 1) * 128, :], out_sb[mc][:])
            for _ in range(24):
                for mk in (nc.sync.nop(), nc.vector.nop(), nc.scalar.nop(),
                           nc.tensor.nop(), nc.gpsimd.nop()):
                    add_dep_helper(mk.ins, last_dma.ins, sync=False,
                                   reason="capw tail carrier")

    return nc


def _cap_waits(nc):
    """This walrus build accepts at most one attached sync wait per
    instruction. Move extra waits onto preceding same-engine instructions
    that (a) have a free wait slot and (b) are positioned after the wait's
    producer in the scheduled order, so no new wait-for cycles can form."""
    for f in nc.m.functions:
        blocks = list(f.blocks)
        glob = []  # (bi, idx, ins) in scheduled order
        for bi, bb in enumerate(blocks):
            for idx, ins in enumerate(bb.instructions):
                glob.append((bi, idx, ins))
        # cumulative semaphore values by global position
        sem_hist = {}  # sem_id -> list of (gpos, cum_after)
        cum = {}
        for g, (bi, idx, ins) in enumerate(glob):
            si = ins.sync_info
            if si is None:
                continue
            for u in si.on_update:
                cum[u.id] = cum.get(u.id, 0) + (u.update_value or 1)
                sem_hist.setdefault(u.id, []).append((g, cum[u.id]))

        def producer_pos(w):
            hist = sem_hist.get(w.id)
            if hist is None:
                return -1
            v = w.wait_value or 1
            for g, c in hist:
                if c >= v:
                    return g
            return len(glob)  # never satisfied: don't move

        # drop waits on the holder's own engine counter: compute engines
        # complete in order, so program order already implies them
        eng_sem = {"EngineType.PE": "PE", "EngineType.DVE": "DVE",
                   "EngineType.Activation": "Activation",
                   "EngineType.Pool": "Pool", "EngineType.SP": "SP"}
        for g, (bi, idx, ins) in enumerate(glob):
            si = ins.sync_info
            if si is None or not si.on_wait:
                continue
            own = eng_sem.get(str(ins.engine))
            kept = [w for w in si.on_wait
                    if not (own and w.ant_name.startswith(own + "_"))]
            if len(kept) != len(si.on_wait):
                ins.sync_info = mybir.SyncInfo(
                    on_wait=kept, on_update=list(si.on_update))

        nwaits = [0 if i.sync_info is None else len(i.sync_info.on_wait)
                  for (_, _, i) in glob]
        eng_pos = {}  # engine -> [gpos...]
        for g, (bi, idx, ins) in enumerate(glob):
            eng_pos.setdefault(str(ins.engine), []).append(g)

        for g, (bi, idx, ins) in enumerate(glob):
            si = ins.sync_info
            if si is None or len(si.on_wait) <= 1:
                continue
            waits = sorted(si.on_wait, key=producer_pos, reverse=True)
            keep = [waits.pop(0)]  # latest-satisfied stays attached
            leftovers = []
            idxs = eng_pos[str(ins.engine)]
            mypos = idxs.index(g)
            for w in waits:
                placed = False
                pmin = producer_pos(w)
                for pp in range(mypos - 1, -1, -1):
                    cg = idxs[pp]
                    if cg <= pmin:
                        break  # earlier carriers are all unsafe
                    cins = glob[cg][2]
                    if nwaits[cg] == 0 and type(cins).__name__ in (
                            "InstNoOp", "InstDrain", "InstMemset", "InstCopy",
                            "InstTensorTensor", "InstActivation",
                            "InstTensorScalarPtr", "InstReciprocal",
                            "InstMatmult", "InstTensorReduce", "InstDMACopy"):
                        csi = cins.sync_info
                        upd = [] if csi is None else list(csi.on_update)
                        cins.sync_info = mybir.SyncInfo(
                            on_wait=[w], on_update=upd)
                        nwaits[cg] = 1
                        placed = True
                        break
                if not placed:
                    leftovers.append(w)
            keep.extend(leftovers)
            ins.sync_info = mybir.SyncInfo(
                on_wait=keep, on_update=list(si.on_update))
            nwaits[g] = len(keep)


@functools.lru_cache(maxsize=1)
def _get_nc():
    nc = _build_nc()
    _cap_waits(nc)
    return nc


@functools.lru_cache(maxsize=1)
def _geom_consts():
    # window mask [98, 1024]: p = 14*wr+wc (chunk ck), qf = 8r+c, tiled x8 tx
    winm = np.zeros((2, 98, 64), dtype=F32)
    for ck in range(2):
        for wr in range(7):
            for wc in range(14):
                for r in range(8):
                    for c in range(8):
                        dy = (7 * ck + wr) - 3 - r
                        dx = wc - 3 - c
                        if abs(dy) <= 3 and abs(dx) <= 3:
                            winm[ck, 14 * wr + wc, 8 * r + c] = 1.0
    winm = np.concatenate([np.tile(winm[ck], (1, 8)) for ck in range(2)], axis=1)
    winm = np.ascontiguousarray(winm).astype(F32)  # [98, 1024]
    e4 = np.zeros((128, 128), dtype=F32)
    for g in range(4):
        e4[g, 32 * g:32 * g + 32] = 1.0
        e4[32 + g, 32 * g:32 * g + 32] = 1.0
    sel = np.zeros((128, 128), dtype=F32)
    sel[0:98, 64] = 1.0
    # npad[y0][q]: # of 7x7 window positions outside the image, per band
    npads = {}
    for y0 in (0, 16, 32, 48):
        npad = np.zeros((2, 8, 8, 8), dtype=F32)  # [ty, tx, r, c]
        for tty in range(2):
            for tx in range(8):
                for r in range(8):
                    for c in range(8):
                        y = y0 + 8 * tty + r
                        x = 8 * tx + c
                        ny = np.arange(y - 3, y + 4)
                        nx = np.arange(x - 3, x + 4)
                        ins_ = ((ny >= 0) & (ny < H))[:, None] & \
                               ((nx >= 0) & (nx < W))[None, :]
                        npad[tty, tx, r, c] = 49 - ins_.sum()
        n128 = np.zeros((128, NQ), dtype=F32)
        n128[0:4] = npad.reshape(1, NQ)
        n128[32:36] = npad.reshape(1, NQ)
        npads[y0] = n128
    return winm, e4, sel, npads


def _host_inputs(queries, keys, values, wq, bq, wk, bk, wv, bv, wo, bo):
    """Build the 8 per-core input maps."""
    winm, e4, sel, npads = _geom_consts()

    # w2 row (32*(h//4) + h%4) = wo[:, head h] @ bv[head h]
    w2 = np.zeros((128, OC), dtype=F32)
    for h in range(HEADS):
        sl = slice(32 * h, 32 * h + 32)
        w2[32 * (h // 4) + h % 4] = wo[:, sl] @ bv[sl]
    e4 = np.zeros((128, 128), dtype=F32)
    for g in range(4):
        e4[g, 32 * g:32 * g + 32] = 1.0
        e4[32 + g, 32 * g:32 * g + 32] = 1.0
    sel = np.zeros((128, 128), dtype=F32)
    sel[0:98, 64] = 1.0

    def wblk(w):  # [256, 256] -> [128p, (kc 2) x 256oc] kc-major
        t = np.ascontiguousarray(w.T).reshape(2, 128, OC)
        return np.concatenate([t[0], t[1]], axis=1)

    biases = np.zeros((128, 6), dtype=F32)
    for t, barr in enumerate((bq, bk, bo)):
        biases[:, 2 * t:2 * t + 2] = barr.reshape(2, 128).T

    in_maps = []
    for core in range(8):
        b, band = core // 4, core % 4
        y0 = band * BAND
        xq = queries[b, :, y0:y0 + BAND, :].reshape(C, 2, 8, 8, 8)
        xq = np.ascontiguousarray(
            xq.transpose(0, 1, 3, 2, 4)).reshape(C, NQ).astype(BF16)
        xk = np.zeros((C, KVR, W), dtype=F32)
        xv = np.zeros((C, KVR, W + 2 * PAD), dtype=F32)
        lo, hi = y0 - PAD, y0 + BAND + PAD
        slo, shi = max(lo, 0), min(hi, H)
        xk[:, slo - lo:shi - lo, :] = keys[b, :, slo:shi, :]
        xv[:, slo - lo:shi - lo, PAD:PAD + W] = values[b, :, slo:shi, :]
        kmask = np.zeros((1, KVR, W), dtype=F32)
        kmask[:, slo - lo:shi - lo, :] = 1.0
        # xv panels: [(ty, ck, tx), wr, wc] window pixels, contiguous per panel
        xvp = np.zeros((C, 2, 2, 8, 7, 14), dtype=F32)
        for tty in range(2):
            for ck in range(2):
                r0 = 8 * tty + 7 * ck
                for tx in range(8):
                    xvp[:, tty, ck, tx] = xv[:, r0:r0 + 7, 8 * tx:8 * tx + 14]
        npad128 = npads[y0]

        cblob = np.zeros((128, CB_N), dtype=F32)
        cblob[:, 0:512] = wblk(wq)
        cblob[:, 512:1024] = wblk(wk)
        cblob[:, 1024:1536] = wblk(wv)
        cblob[:, 1536:2048] = wblk(wo)
        cblob[:, 2048:2304] = w2
        cblob[:, 2304:2304 + NKV] = np.broadcast_to(kmask.reshape(1, NKV), (128, NKV))
        cblob[0:98, 3712:4736] = winm
        cblob[:, 4736:5760] = npad128
        cblob[:, 5760:5888] = e4
        cblob[:, 5888:6016] = sel
        in_maps.append(dict(
            xq=xq,
            xk=xk.reshape(C, NKV).astype(BF16),
            xv=xvp.reshape(C, 2 * 2 * 8 * 98).astype(BF16),
            cblob=cblob.astype(BF16),
            biases=biases,
        ))
    return in_maps


def kernel(queries, keys, values, wq, bq, wk, bk, wv, bv, wo, bo):
    global LAST_EXEC_NS
    nc = _get_nc()
    in_maps = _host_inputs(queries, keys, values, wq, bq, wk, bk, wv, bv, wo, bo)
    trace = bool(os.environ.get("KERNEL_TRACE"))
    try:
        res = run_bass_kernel_spmd(nc, in_maps, core_ids=list(range(8)),
                                   trace=trace)
    except ModuleNotFoundError:
        # NTFF profile hook unavailable in this container
        res = run_bass_kernel_spmd(nc, in_maps, core_ids=list(range(8)),
                                   trace=False)
    LAST_EXEC_NS = res.exec_time_ns
    out = np.zeros((B, OC, H, W), dtype=F32)
    for core in range(8):
        b, band = core // 4, core % 4
        y0 = band * BAND
        out[b, :, y0:y0 + BAND, :] = res.results[core]["out"].reshape(OC, BAND, W)
    return out
